# revision 11
# baseline (speedup 1.0000x reference)
"""Multi-head attention (B=2, T=2048, C=1024, H=16, hd=64, RoPE, full mask)
on 8 TRN2 NeuronCores.

Sharding: tensor-parallel over (batch, head-group). Core c handles batch
c//4 and heads [4*(c%4) .. 4*(c%4)+3]. Each core computes the QKV
projection for its 4 heads, full attention over T=2048, and a partial
output projection y = o_heads @ w_proj[:, cols].T. The host sums the 4
partial y's per batch (the tensor-parallel unshard reduction).

On-chip layout is "transposed everything" so attention needs no on-chip
transposes:
  - qT/kT stored [head_dim, T] (projection computed as w @ x.T)
  - scores computed directly transposed: sT[tk, tq] = k[tk] . q[tq]
  - softmax denominator via an appended ones-column on V (M=66 matmul)
  - o.T scaled by 1/den with a K=1 outer-product broadcast matmul
RoPE is applied with full-width elementwise ops after a host-side
even/odd row permutation of w_q/w_k plus one PE permutation matmul that
swaps adjacent 32-partition blocks.

Matmuls run in float32r (1 PE cycle/column at N>=256 vs 4 for fp32,
~1e-4 relative precision measured on HW). PSUM accumulation is fp32.
"""

import numpy as np

import concourse.bacc as bacc
import concourse.mybir as mybir
import concourse.tile as tile
from concourse.bass_utils import run_bass_kernel_spmd

# Problem constants (hardcoded per contract)
B, T, C = 2, 2048, 1024
N_HEAD = 16
HD = 64
HH = HD // 2  # 32
N_CORES = 8
HPC = 4  # heads per core
GC = HPC * HD  # head channels per core = 256

P = 128
KC = C // P  # 8 contraction chunks for the projections
NQB = 4  # query blocks
TQ = T // NQB  # 512
NKB = T // P  # 16 key blocks
VW = HD + 2  # 66: v + ones col + pad col (fp32r needs even free count)

F32 = mybir.dt.float32
F32R = mybir.dt.float32r

_PROGRAM = None


def _build_program():
    nc = bacc.Bacc(
        "TRN2", target_bir_lowering=False, debug=False, num_devices=N_CORES
    )

    xT_d = nc.dram_tensor("xT", [C, T], F32R, kind="ExternalInput").ap()
    wqkT_d = nc.dram_tensor("wqkT", [C, 4 * P], F32R, kind="ExternalInput").ap()
    wvT_d = nc.dram_tensor("wvT", [C, GC], F32R, kind="ExternalInput").ap()
    wpT_d = nc.dram_tensor("wpT", [GC, C], F32R, kind="ExternalInput").ap()
    cc_d = nc.dram_tensor("cc", [P, T], F32, kind="ExternalInput").ap()
    ss_d = nc.dram_tensor("ss", [P, T], F32, kind="ExternalInput").ap()
    psw_d = nc.dram_tensor("psw", [P, P], F32R, kind="ExternalInput").ap()
    y_d = nc.dram_tensor("y", [T, C], F32, kind="ExternalOutput").ap()

    with tile.TileContext(nc) as tc:
        with (
            tc.tile_pool(name="consts", bufs=1) as consts,
            tc.tile_pool(name="bigs", bufs=1) as bigs,
            tc.tile_pool(name="tmps", bufs=2) as tmps,
            tc.tile_pool(name="expool", bufs=3) as expool,
            tc.tile_pool(name="psA", bufs=4, space="PSUM") as psA,
            tc.tile_pool(name="psB", bufs=2, space="PSUM") as psB,
            tc.tile_pool(name="psC", bufs=2, space="PSUM") as psC,
        ):
            # ---- resident loads ----
            x_k = []
            for kc in range(KC):
                t = bigs.tile([P, T], F32R, tag=f"x{kc}", name=f"x{kc}")
                nc.sync.dma_start(out=t, in_=xT_d[kc * P : (kc + 1) * P, :])
                x_k.append(t)
            wqk_k = []
            for kc in range(KC):
                t = bigs.tile([P, 4 * P], F32R, tag=f"wqk{kc}", name=f"wqk{kc}")
                nc.sync.dma_start(out=t, in_=wqkT_d[kc * P : (kc + 1) * P, :])
                wqk_k.append(t)
            wv_k = []
            for kc in range(KC):
                t = bigs.tile([P, GC], F32R, tag=f"wv{kc}", name=f"wv{kc}")
                nc.sync.dma_start(out=t, in_=wvT_d[kc * P : (kc + 1) * P, :])
                wv_k.append(t)
            wp_k = []
            for kb in range(2):
                t = bigs.tile([P, C], F32R, tag=f"wp{kb}", name=f"wp{kb}")
                nc.sync.dma_start(out=t, in_=wpT_d[kb * P : (kb + 1) * P, :])
                wp_k.append(t)
            cc_t = consts.tile([P, T], F32, tag="cc")
            nc.sync.dma_start(out=cc_t, in_=cc_d)
            ss_t = consts.tile([P, T], F32, tag="ss")
            nc.sync.dma_start(out=ss_t, in_=ss_d)
            psw_t = consts.tile([P, P], F32R, tag="psw")
            nc.sync.dma_start(out=psw_t, in_=psw_d)
            ones_f = consts.tile([P, 2 * HPC], F32, tag="ones_f")
            nc.vector.memset(ones_f, 1.0)
            ones64f = consts.tile([1, HD], F32, tag="ones64f")
            nc.vector.memset(ones64f, 1.0)
            ones64r = consts.tile([1, HD], F32R, tag="ones64r")
            nc.vector.tensor_copy(ones64r, ones64f)

            # ---- phase 1a: q/k projection + RoPE ----
            # M-blocks: 0 -> heads 0,1 of q; 1 -> heads 2,3 of q;
            #           2 -> heads 0,1 of k; 3 -> heads 2,3 of k.
            # Within a block: [h_even rows(32); h_odd(32); h'_even; h'_odd]
            qk_sb = [
                bigs.tile([P, T], F32R, tag=f"qk{mb}", name=f"qk{mb}")
                for mb in range(4)
            ]
            for n in range(NQB):
                ns = slice(n * TQ, (n + 1) * TQ)
                for mb in range(4):
                    ps = psA.tile([P, TQ], F32, tag="mmps", name="ps")
                    for kc in range(KC):
                        nc.tensor.matmul(
                            ps,
                            lhsT=wqk_k[kc][:, mb * P : (mb + 1) * P],
                            rhs=x_k[kc][:, ns],
                            start=(kc == 0),
                            stop=(kc == KC - 1),
                        )
                    sb = qk_sb[mb]
                    nc.vector.tensor_copy(sb[:, ns], ps)
                    sw = psB.tile([P, TQ], F32, tag="aux", name="sw")
                    nc.tensor.matmul(
                        sw, lhsT=psw_t, rhs=sb[:, ns], start=True, stop=True
                    )
                    nc.vector.tensor_mul(sb[:, ns], sb[:, ns], cc_t[:, ns])
                    tmp = tmps.tile([P, TQ], F32, tag="ropetmp")
                    nc.vector.tensor_mul(tmp, sw, ss_t[:, ns])
                    nc.vector.tensor_add(sb[:, ns], sb[:, ns], tmp)

            # ---- phase 1b: v projection into [T, 4*66] with ones cols ----
            va_list = []
            for tb in range(NKB):
                vp = psA.tile([P, GC], F32, tag="mmps", name="vp")
                for kc in range(KC):
                    nc.tensor.matmul(
                        vp,
                        lhsT=x_k[kc][:, tb * P : (tb + 1) * P],
                        rhs=wv_k[kc],
                        start=(kc == 0),
                        stop=(kc == KC - 1),
                    )
                va = bigs.tile([P, HPC * VW], F32R, tag=f"va{tb}", name=f"va{tb}")
                va4 = va.rearrange("p (h c) -> p h c", c=VW)
                nc.vector.tensor_copy(
                    va4[:, :, HD : HD + 2],
                    ones_f.rearrange("p (h c) -> p h c", c=2),
                )
                nc.vector.tensor_copy(
                    va4[:, :, 0:HD], vp.rearrange("p (h c) -> p h c", c=HD)
                )
                va_list.append(va)

            # ---- phases 2+3 per query block ----
            for qb in range(NQB):
                qs = slice(qb * TQ, (qb + 1) * TQ)
                o_sb = [
                    tmps.tile(
                        [P, TQ], F32R, tag=f"osb{kb}", name=f"osb{kb}_{qb}",
                        bufs=2,
                    )
                    for kb in range(2)
                ]
                for h in range(HPC):
                    qt = qk_sb[h // 2][(h % 2) * HD : (h % 2) * HD + HD, :]
                    kt = qk_sb[2 + h // 2][(h % 2) * HD : (h % 2) * HD + HD, :]
                    oau = psC.tile([VW, TQ], F32, tag="oau", name="oau")
                    for kb in range(NKB):
                        st = psA.tile([P, TQ], F32, tag="mmps", name="st")
                        nc.tensor.matmul(
                            st,
                            lhsT=kt[:, kb * P : (kb + 1) * P],
                            rhs=qt[:, qs],
                            start=True,
                            stop=True,
                        )
                        ex = expool.tile([P, TQ], F32R, tag="ex", name="ex")
                        nc.scalar.activation(
                            out=ex,
                            in_=st,
                            func=mybir.ActivationFunctionType.Exp,
                            scale=1.0 / np.sqrt(HD),
                        )
                        nc.tensor.matmul(
                            oau,
                            lhsT=va_list[kb][:, h * VW : h * VW + VW],
                            rhs=ex,
                            start=(kb == 0),
                            stop=(kb == NKB - 1),
                        )
                    rden = tmps.tile([1, TQ], F32R, tag="rden")
                    with nc.allow_low_precision(reason="f32r round of 1/den"):
                        nc.vector.reciprocal(rden, oau[HD : HD + 1, :])
                    bc = psB.tile([HD, TQ], F32, tag="aux", name="bc")
                    nc.tensor.matmul(
                        bc, lhsT=ones64r, rhs=rden, start=True, stop=True
                    )
                    oev = tmps.tile([HD, TQ], F32, tag="oev")
                    nc.vector.tensor_copy(oev, oau[0:HD, :])
                    half = (h % 2) * HD
                    nc.vector.tensor_mul(
                        o_sb[h // 2][half : half + HD, :], oev, bc
                    )

                # output projection for this query block: y rows qb*512..
                for tch in range(TQ // P):
                    for cch in range(C // TQ):
                        yp = psA.tile([P, TQ], F32, tag="mmps", name="yp")
                        for kb in range(2):
                            nc.tensor.matmul(
                                yp,
                                lhsT=o_sb[kb][:, tch * P : (tch + 1) * P],
                                rhs=wp_k[kb][:, cch * TQ : (cch + 1) * TQ],
                                start=(kb == 0),
                                stop=(kb == 1),
                            )
                        ysb = tmps.tile([P, TQ], F32, tag="ysb")
                        nc.vector.tensor_copy(ysb, yp)
                        r0 = qb * TQ + tch * P
                        nc.sync.dma_start(
                            out=y_d[r0 : r0 + P, cch * TQ : (cch + 1) * TQ],
                            in_=ysb,
                        )

    nc.compile()
    return nc


def _get_program():
    global _PROGRAM
    if _PROGRAM is None:
        _PROGRAM = _build_program()
    return _PROGRAM


def _eo(w):
    """[64, C] head rows -> [even(32); odd(32)]"""
    return np.concatenate([w[0::2], w[1::2]], axis=0)


def _host_prep(x, cos, sin, w_qkv, w_proj):
    """Build the 8 per-core input maps."""
    xT = [np.ascontiguousarray(x[b].T) for b in range(B)]  # [C, T]

    cosT = np.ascontiguousarray(cos.T)  # [32, T]
    sinT = np.ascontiguousarray(sin.T)
    cc = np.tile(cosT, (4, 1)).astype(np.float32)  # [128, T]
    ss = np.tile(
        np.concatenate([-sinT, sinT], axis=0), (2, 1)
    ).astype(np.float32)
    psw = np.zeros((P, P), dtype=np.float32)
    idx = np.arange(P)
    psw[idx, idx ^ 32] = 1.0

    wq = w_qkv[0:C]
    wk = w_qkv[C : 2 * C]
    wv = w_qkv[2 * C : 3 * C]

    in_maps = []
    for core in range(N_CORES):
        b = core // 4
        h0 = 4 * (core % 4)
        heads = [h0, h0 + 1, h0 + 2, h0 + 3]
        blocks = []
        for pair in range(2):
            ha, hb = heads[2 * pair], heads[2 * pair + 1]
            blocks.append(
                np.concatenate(
                    [_eo(wq[ha * HD : ha * HD + HD]),
                     _eo(wq[hb * HD : hb * HD + HD])],
                    axis=0,
                )
            )
        for pair in range(2):
            ha, hb = heads[2 * pair], heads[2 * pair + 1]
            blocks.append(
                np.concatenate(
                    [_eo(wk[ha * HD : ha * HD + HD]),
                     _eo(wk[hb * HD : hb * HD + HD])],
                    axis=0,
                )
            )
        wqkT = np.ascontiguousarray(
            np.concatenate(blocks, axis=0).T
        ).astype(np.float32)  # [C, 512]
        wvT = np.ascontiguousarray(
            wv[h0 * HD : h0 * HD + GC].T
        ).astype(np.float32)  # [C, 256]
        wpT = np.ascontiguousarray(
            w_proj[:, h0 * HD : h0 * HD + GC].T
        ).astype(np.float32)  # [256, C]
        in_maps.append(
            {
                "xT": xT[b],
                "wqkT": wqkT,
                "wvT": wvT,
                "wpT": wpT,
                "cc": cc,
                "ss": ss,
                "psw": psw,
            }
        )
    return in_maps


def kernel(x, cos, sin, mask, w_qkv, w_proj, _trace=False, _tmpdir=None):
    x = np.asarray(x, dtype=np.float32)
    cos = np.asarray(cos, dtype=np.float32)
    sin = np.asarray(sin, dtype=np.float32)
    w_qkv = np.asarray(w_qkv, dtype=np.float32)
    w_proj = np.asarray(w_proj, dtype=np.float32)
    # mask is all-ones in this problem spec: no-op in the math.

    nc = _get_program()
    in_maps = _host_prep(x, cos, sin, w_qkv, w_proj)
    res = run_bass_kernel_spmd(
        nc, in_maps, list(range(N_CORES)), trace=_trace, tmpdir=_tmpdir
    )
    out = np.empty((B, T, C), dtype=np.float32)
    for b in range(B):
        acc = res.results[4 * b]["y"].astype(np.float32).copy()
        for g in range(1, 4):
            acc += res.results[4 * b + g]["y"]
        out[b] = acc
    kernel._last_exec_time_ns = res.exec_time_ns
    return out


# revision 15
# speedup vs baseline: 1.8093x; 1.8093x over previous
"""Multi-head attention (B=2, T=2048, C=1024, H=16, hd=64, RoPE, full mask)
on 8 TRN2 NeuronCores.

Sharding: tensor-parallel over (batch, head-group). Core c handles batch
c//4 and heads [4*(c%4) .. 4*(c%4)+3]. Each core computes the QKV
projection for its 4 heads, full attention over T=2048, and a partial
output projection y = o_heads @ w_proj[:, cols].T. The host sums the 4
partial y's per batch (the tensor-parallel unshard reduction).

On-chip layout is "transposed everything" so attention needs no on-chip
transposes of the big tensors:
  - qT/kT stored [head_dim, T] (projection computed as w @ x.T)
  - scores computed directly transposed: sT[tk, tq] = k[tk] . q[tq]
  - softmax denominator via an appended ones-column on V (M=66 matmul)
  - o.T scaled by 1/den via a small PE-transpose + batched reciprocal +
    0/1-matrix broadcast matmul
RoPE uses full-width elementwise ops after a host-side even/odd row
permutation of w_q/w_k plus one PE permutation matmul that swaps
adjacent 32-partition blocks.

Precision: projections (K=1024/256 contractions) run in float32r
(~1e-4); attention q/k/v/softmax run in bf16 with fp32 PSUM
accumulation. Heads are processed in pairs so each exp ACTIVATE covers
[128, 1024] (amortizes the ~200ns ACT op overhead).
"""

import ml_dtypes
import numpy as np

import concourse.bacc as bacc
import concourse.mybir as mybir
import concourse.tile as tile
from concourse.bass_utils import run_bass_kernel_spmd

# Problem constants (hardcoded per contract)
B, T, C = 2, 2048, 1024
N_HEAD = 16
HD = 64
N_CORES = 8
HPC = 4  # heads per core
GC = HPC * HD  # head channels per core = 256

P = 128
KC = C // P  # 8 contraction chunks for the projections
NQB = 4  # query blocks
TQ = T // NQB  # 512
NKB = T // P  # 16 key blocks
VW = HD + 2  # 66: v + ones col + pad col

F32 = mybir.dt.float32
F32R = mybir.dt.float32r
BF16 = mybir.dt.bfloat16

_PROGRAM = None


def _build_program():
    nc = bacc.Bacc(
        "TRN2", target_bir_lowering=False, debug=False, num_devices=N_CORES
    )

    xT_d = nc.dram_tensor("xT", [C, T], F32R, kind="ExternalInput").ap()
    wqkT_d = nc.dram_tensor("wqkT", [C, 4 * P], F32R, kind="ExternalInput").ap()
    wvT_d = nc.dram_tensor("wvT", [C, GC], F32R, kind="ExternalInput").ap()
    wpT_d = nc.dram_tensor("wpT", [GC, C], F32R, kind="ExternalInput").ap()
    cc_d = nc.dram_tensor("cc", [P, T], BF16, kind="ExternalInput").ap()
    ss_d = nc.dram_tensor("ss", [P, T], BF16, kind="ExternalInput").ap()
    psw_d = nc.dram_tensor("psw", [P, P], BF16, kind="ExternalInput").ap()
    emat_d = nc.dram_tensor("emat", [HPC, 2 * P], F32R, kind="ExternalInput").ap()
    ident_d = nc.dram_tensor("ident", [P, P], F32, kind="ExternalInput").ap()
    esel_d = nc.dram_tensor("esel", [P, HPC], F32, kind="ExternalInput").ap()
    y_d = nc.dram_tensor("y", [T, C], F32, kind="ExternalOutput").ap()

    with tile.TileContext(nc) as tc:
        with (
            tc.tile_pool(name="consts", bufs=1) as consts,
            tc.tile_pool(name="bigs", bufs=1) as bigs,
            tc.tile_pool(name="tmps", bufs=2) as tmps,
            tc.tile_pool(name="expool", bufs=3) as expool,
            tc.tile_pool(name="psA", bufs=2, space="PSUM") as psA,
            tc.tile_pool(name="psB", bufs=2, space="PSUM") as psB,
            tc.tile_pool(name="psC", bufs=2, space="PSUM") as psC,
        ):
            # ---- resident loads ----
            x_k = []
            for kc in range(KC):
                t = bigs.tile([P, T], F32R, tag=f"x{kc}", name=f"x{kc}")
                nc.sync.dma_start(out=t, in_=xT_d[kc * P : (kc + 1) * P, :])
                x_k.append(t)
            wqk_k = []
            for kc in range(KC):
                t = bigs.tile([P, 4 * P], F32R, tag=f"wqk{kc}", name=f"wqk{kc}")
                nc.sync.dma_start(out=t, in_=wqkT_d[kc * P : (kc + 1) * P, :])
                wqk_k.append(t)
            wv_k = []
            for kc in range(KC):
                t = bigs.tile([P, GC], F32R, tag=f"wv{kc}", name=f"wv{kc}")
                nc.sync.dma_start(out=t, in_=wvT_d[kc * P : (kc + 1) * P, :])
                wv_k.append(t)
            wp_k = []
            for kb in range(2):
                t = bigs.tile([P, C], F32R, tag=f"wp{kb}", name=f"wp{kb}")
                nc.sync.dma_start(out=t, in_=wpT_d[kb * P : (kb + 1) * P, :])
                wp_k.append(t)
            cc_t = consts.tile([P, T], BF16, tag="cc")
            nc.sync.dma_start(out=cc_t, in_=cc_d)
            ss_t = consts.tile([P, T], BF16, tag="ss")
            nc.sync.dma_start(out=ss_t, in_=ss_d)
            psw_t = consts.tile([P, P], BF16, tag="psw")
            nc.sync.dma_start(out=psw_t, in_=psw_d)
            emat_t = consts.tile([HPC, 2 * P], F32R, tag="emat")
            nc.sync.dma_start(out=emat_t, in_=emat_d)
            ident_t = consts.tile([P, P], F32, tag="ident")
            nc.sync.dma_start(out=ident_t, in_=ident_d)
            esel_t = consts.tile([P, HPC], F32, tag="esel")
            nc.sync.dma_start(out=esel_t, in_=esel_d)
            ones_f = consts.tile([P, 2 * HPC], F32, tag="ones_f")
            nc.vector.memset(ones_f, 1.0)

            # ---- phase 1a: q/k projection + RoPE (output bf16) ----
            # M-blocks: 0 -> heads 0,1 of q; 1 -> heads 2,3 of q;
            #           2 -> heads 0,1 of k; 3 -> heads 2,3 of k.
            # Within a block: [h_even rows(32); h_odd(32); h'_even; h'_odd]
            qk_sb = [
                bigs.tile([P, T], BF16, tag=f"qk{mb}", name=f"qk{mb}")
                for mb in range(4)
            ]
            for n in range(NQB):
                ns = slice(n * TQ, (n + 1) * TQ)
                for mb in range(4):
                    ps = psA.tile([P, 2 * TQ], F32, tag="mmps", name="ps")
                    pss = ps[:, 0:TQ]
                    for kc in range(KC):
                        nc.tensor.matmul(
                            pss,
                            lhsT=wqk_k[kc][:, mb * P : (mb + 1) * P],
                            rhs=x_k[kc][:, ns],
                            start=(kc == 0),
                            stop=(kc == KC - 1),
                        )
                    sb = qk_sb[mb]
                    nc.vector.tensor_copy(sb[:, ns], pss)
                    sw = psB.tile([P, TQ], F32, tag="aux", name="sw")
                    nc.tensor.matmul(
                        sw, lhsT=psw_t, rhs=sb[:, ns], start=True, stop=True
                    )
                    nc.vector.tensor_mul(sb[:, ns], sb[:, ns], cc_t[:, ns])
                    tmp = tmps.tile([P, TQ], BF16, tag="ropetmp")
                    nc.vector.tensor_mul(tmp, sw, ss_t[:, ns])
                    nc.vector.tensor_add(sb[:, ns], sb[:, ns], tmp)

            # ---- phase 1b: v projection into [T, 4*66] bf16, ones col ----
            va_list = []
            for tb in range(NKB):
                vp = psA.tile([P, 2 * TQ], F32, tag="mmps", name="vp")
                vps = vp[:, 0:GC]
                for kc in range(KC):
                    nc.tensor.matmul(
                        vps,
                        lhsT=x_k[kc][:, tb * P : (tb + 1) * P],
                        rhs=wv_k[kc],
                        start=(kc == 0),
                        stop=(kc == KC - 1),
                    )
                va = bigs.tile(
                    [P, HPC * VW], BF16, tag=f"va{tb}", name=f"va{tb}"
                )
                va4 = va.rearrange("p (h c) -> p h c", c=VW)
                nc.vector.tensor_copy(
                    va4[:, :, HD : HD + 2],
                    ones_f.rearrange("p (h c) -> p h c", c=2),
                )
                nc.vector.tensor_copy(
                    va4[:, :, 0:HD], vps.rearrange("p (h c) -> p h c", c=HD)
                )
                va_list.append(va)

            # ---- phases 2+3 per query block ----
            for qb in range(NQB):
                qs = slice(qb * TQ, (qb + 1) * TQ)
                o_sb = [
                    tmps.tile(
                        [P, TQ], F32R, tag=f"osb{p}", name=f"osb{p}_{qb}",
                        bufs=2,
                    )
                    for p in range(2)
                ]
                oevp = [
                    tmps.tile(
                        [P, TQ], F32, tag=f"oevp{p}", name=f"oevp{p}_{qb}",
                        bufs=2,
                    )
                    for p in range(2)
                ]
                den4 = tmps.tile([P, TQ], F32, tag="den4", name=f"den4_{qb}")
                nc.vector.memset(den4, 1.0)
                for p in range(2):
                    qt = qk_sb[p]
                    kt = qk_sb[2 + p]
                    oau = [
                        psC.tile([VW, TQ], F32, tag="oau", name=f"oau{i}")
                        for i in range(2)
                    ]
                    # software pipeline: AV lags QK/exp by one k-block so
                    # the PE never stalls waiting on the exp
                    st2s = {}
                    exs = {}
                    for kb in range(NKB + 1):
                        if kb < NKB:
                            st2 = psA.tile(
                                [P, 2 * TQ], F32, tag="mmps", name="st2"
                            )
                            ks = slice(kb * P, (kb + 1) * P)
                            for i in range(2):
                                nc.tensor.matmul(
                                    st2[:, i * TQ : (i + 1) * TQ],
                                    lhsT=kt[i * HD : (i + 1) * HD, ks],
                                    rhs=qt[i * HD : (i + 1) * HD, qs],
                                    start=True,
                                    stop=True,
                                )
                            ex = expool.tile(
                                [P, 2 * TQ], BF16, tag="ex", name="ex"
                            )
                            nc.scalar.activation(
                                out=ex,
                                in_=st2,
                                func=mybir.ActivationFunctionType.Exp,
                                scale=1.0 / np.sqrt(HD),
                            )
                            st2s[kb] = st2
                            exs[kb] = ex
                        if kb >= 1:
                            pk = kb - 1
                            exp_prev = exs.pop(pk)
                            st2s.pop(pk)
                            for i in range(2):
                                h = 2 * p + i
                                nc.tensor.matmul(
                                    oau[i],
                                    lhsT=va_list[pk][:, h * VW : h * VW + VW],
                                    rhs=exp_prev[:, i * TQ : (i + 1) * TQ],
                                    start=(pk == 0),
                                    stop=(pk == NKB - 1),
                                )
                    # stage o (unnormalized) and the denominators
                    for i in range(2):
                        nc.vector.tensor_copy(
                            oevp[p][i * HD : (i + 1) * HD, :], oau[i][0:HD, :]
                        )
                        r = 32 * (2 * p + i)
                        nc.vector.tensor_copy(
                            den4[r : r + 1, :], oau[i][HD : HD + 1, :]
                        )

                # batched reciprocal: transpose den4 to [128, 16], one
                # reciprocal, transpose back, broadcast via 0/1 matmul
                denT = psB.tile([P, 4 * HPC], F32, tag="aux", name="denT")
                for c in range(4):
                    nc.tensor.matmul(
                        denT[:, c * HPC : (c + 1) * HPC],
                        lhsT=den4[:, c * P : (c + 1) * P],
                        rhs=esel_t,
                        start=True,
                        stop=True,
                    )
                rdenT = tmps.tile([P, 4 * HPC], F32, tag="rdenT")
                nc.vector.reciprocal(rdenT, denT)
                rden_ps = psB.tile([HPC, TQ], F32, tag="aux", name="rden_ps")
                for c in range(4):
                    nc.tensor.transpose(
                        rden_ps[:, c * P : (c + 1) * P],
                        rdenT[:, c * HPC : (c + 1) * HPC],
                        ident_t,
                    )
                rden4 = tmps.tile([HPC, TQ], F32R, tag="rden4")
                with nc.allow_low_precision(reason="f32r round of 1/den"):
                    nc.vector.tensor_copy(rden4, rden_ps)
                for p in range(2):
                    bc = psB.tile([P, TQ], F32, tag="aux", name="bc")
                    nc.tensor.matmul(
                        bc,
                        lhsT=emat_t[:, p * P : (p + 1) * P],
                        rhs=rden4,
                        start=True,
                        stop=True,
                    )
                    nc.vector.tensor_mul(o_sb[p], oevp[p], bc)

                # output projection for this query block: y rows qb*512..
                for tch in range(TQ // P):
                    for cch in range(C // TQ):
                        yp = psB.tile([P, TQ], F32, tag="aux", name="yp")
                        for kb in range(2):
                            nc.tensor.matmul(
                                yp,
                                lhsT=o_sb[kb][:, tch * P : (tch + 1) * P],
                                rhs=wp_k[kb][:, cch * TQ : (cch + 1) * TQ],
                                start=(kb == 0),
                                stop=(kb == 1),
                            )
                        ysb = tmps.tile([P, TQ], F32, tag="ysb")
                        nc.vector.tensor_copy(ysb, yp)
                        r0 = qb * TQ + tch * P
                        nc.sync.dma_start(
                            out=y_d[r0 : r0 + P, cch * TQ : (cch + 1) * TQ],
                            in_=ysb,
                        )

    nc.compile()
    return nc


def _get_program():
    global _PROGRAM
    if _PROGRAM is None:
        _PROGRAM = _build_program()
    return _PROGRAM


def _eo(w):
    """[64, C] head rows -> [even(32); odd(32)]"""
    return np.concatenate([w[0::2], w[1::2]], axis=0)


def _host_prep(x, cos, sin, w_qkv, w_proj):
    """Build the 8 per-core input maps."""
    bf16 = ml_dtypes.bfloat16
    xT = [np.ascontiguousarray(x[b].T) for b in range(B)]  # [C, T]

    cosT = np.ascontiguousarray(cos.T)  # [32, T]
    sinT = np.ascontiguousarray(sin.T)
    cc = np.tile(cosT, (4, 1)).astype(bf16)  # [128, T]
    ss = np.tile(np.concatenate([-sinT, sinT], axis=0), (2, 1)).astype(bf16)
    psw = np.zeros((P, P), dtype=np.float32)
    idx = np.arange(P)
    psw[idx, idx ^ 32] = 1.0
    psw = psw.astype(bf16)
    emat = np.zeros((HPC, 2 * P), dtype=np.float32)
    for p in range(2):
        for i in range(2):
            emat[2 * p + i, p * P + i * HD : p * P + (i + 1) * HD] = 1.0
    ident = np.eye(P, dtype=np.float32)
    esel = np.zeros((P, HPC), dtype=np.float32)
    for j in range(HPC):
        esel[32 * j, j] = 1.0

    wq = w_qkv[0:C]
    wk = w_qkv[C : 2 * C]
    wv = w_qkv[2 * C : 3 * C]

    in_maps = []
    for core in range(N_CORES):
        b = core // 4
        h0 = 4 * (core % 4)
        heads = [h0, h0 + 1, h0 + 2, h0 + 3]
        blocks = []
        for pair in range(2):
            ha, hb = heads[2 * pair], heads[2 * pair + 1]
            blocks.append(
                np.concatenate(
                    [_eo(wq[ha * HD : ha * HD + HD]),
                     _eo(wq[hb * HD : hb * HD + HD])],
                    axis=0,
                )
            )
        for pair in range(2):
            ha, hb = heads[2 * pair], heads[2 * pair + 1]
            blocks.append(
                np.concatenate(
                    [_eo(wk[ha * HD : ha * HD + HD]),
                     _eo(wk[hb * HD : hb * HD + HD])],
                    axis=0,
                )
            )
        wqkT = np.ascontiguousarray(
            np.concatenate(blocks, axis=0).T
        ).astype(np.float32)  # [C, 512]
        wvT = np.ascontiguousarray(
            wv[h0 * HD : h0 * HD + GC].T
        ).astype(np.float32)  # [C, 256]
        wpT = np.ascontiguousarray(
            w_proj[:, h0 * HD : h0 * HD + GC].T
        ).astype(np.float32)  # [256, C]
        in_maps.append(
            {
                "xT": xT[b],
                "wqkT": wqkT,
                "wvT": wvT,
                "wpT": wpT,
                "cc": cc,
                "ss": ss,
                "psw": psw,
                "emat": emat,
                "ident": ident,
                "esel": esel,
            }
        )
    return in_maps


def kernel(x, cos, sin, mask, w_qkv, w_proj, _trace=False, _tmpdir=None):
    x = np.asarray(x, dtype=np.float32)
    cos = np.asarray(cos, dtype=np.float32)
    sin = np.asarray(sin, dtype=np.float32)
    w_qkv = np.asarray(w_qkv, dtype=np.float32)
    w_proj = np.asarray(w_proj, dtype=np.float32)
    # mask is all-ones in this problem spec: no-op in the math.

    nc = _get_program()
    in_maps = _host_prep(x, cos, sin, w_qkv, w_proj)
    res = run_bass_kernel_spmd(
        nc, in_maps, list(range(N_CORES)), trace=_trace, tmpdir=_tmpdir
    )
    out = np.empty((B, T, C), dtype=np.float32)
    for b in range(B):
        acc = res.results[4 * b]["y"].astype(np.float32).copy()
        for g in range(1, 4):
            acc += res.results[4 * b + g]["y"]
        out[b] = acc
    kernel._last_exec_time_ns = res.exec_time_ns
    return out
